# revision 80
# baseline (speedup 1.0000x reference)
"""Trainium2 Bass kernel for a single-layer MHA + FFN transformer block.

Reference computation (x: [1, 4096, 768], 12 heads, dff=3072):
    qkv = (x @ w_qkv + b_qkv)  -> q, k, v
    scores = q k^T / sqrt(768) ; wei = softmax(scores)
    attn = wei @ v  (concat heads)
    h = gelu(attn @ w_ff1 + b_ff1)
    out = h @ w_ff2 + b_ff2

Sharding: a 4 (head-groups) x 2 (sequence-halves) grid over 8 cores.
Core c = 2h + s owns heads 3h..3h+3 and tokens 2048s..2048(s+1). Each
core projects q/k/v for its own heads+tokens, exchanges k/v with its
sequence partner via a masked pair ReduceScatter (host-provided 0/1
masks make the SPMD program rank-independent: each core contributes
[kv*m0 | kv*m1] with m = [s, 1-s], so the add-reduction delivers
exactly the partner's kv), runs attention for its 3 heads over all
4096 keys (local keys first so the collective is hidden), then an
8-core AllToAll (same masking idea, parity masks) redistributes
attention outputs so every core holds 512 full-width rows for the
row-parallel FFN.

Precision: FFN matmuls run in fp32r; projections and the attention
triangle run in bf16 with fp32 PSUM accumulation; softmax skips
max-subtraction because the logits are bounded (~0.6); softmax
denominators come free via a ones-column appended to v.
"""

import json as _json
import math

import numpy as np

import concourse.bass as bass
import concourse.mybir as mybir
import concourse.tile as tile
from concourse.bass_utils import run_bass_kernel_spmd

# ---------------------------------------------------------------------------
# Workaround: the pinned walrus build only supports ONE embedded semaphore
# wait per instruction, but Tile's sem assigner attaches several. Split the
# excess onto standalone EventSemaphore instructions (pure waits) inserted
# just before the over-subscribed instruction (same engine => same program
# order, identical semantics).
# ---------------------------------------------------------------------------
_MAX_WAITS = 1
_ctr = [0]
if not getattr(bass.Bass, "_multiwait_patched", False):
    _orig_to_json_bytes = bass.Bass.to_json_bytes

    def _split_multiwait_json_bytes(self):
        bir = _json.loads(_orig_to_json_bytes(self))
        for f in bir["functions"]:
            for b in f["blocks"]:
                new_insts = []
                for inst in b["instructions"]:
                    si = inst.get("sync_info")
                    waits = si.get("on_wait", []) if si else []
                    if len(waits) > _MAX_WAITS:
                        excess, keep = waits[:-_MAX_WAITS], waits[-_MAX_WAITS:]
                        for k in range(0, len(excess), _MAX_WAITS):
                            _ctr[0] += 1
                            new_insts.append({
                                "debug": inst.get("debug", 0),
                                "engine": inst["engine"],
                                "ins": [], "outs": [],
                                "name": "I-waitsplit-%d" % _ctr[0],
                                "opcode": "EventSemaphore",
                                "sync_info": {"on_update": [],
                                              "on_wait": excess[k:k + _MAX_WAITS]},
                            })
                        si["on_wait"] = keep
                    new_insts.append(inst)
                b["instructions"] = new_insts
        return _json.dumps(bir).encode()

    bass.Bass.to_json_bytes = _split_multiwait_json_bytes
    bass.Bass._multiwait_patched = True

F32 = mybir.dt.float32
F32R = mybir.dt.float32r
BF16 = mybir.dt.bfloat16
AFT = mybir.ActivationFunctionType
ALU = mybir.AluOpType

R = 8            # cores
T = 4096         # sequence length
TLOC = 2048      # tokens per core (2 seq groups)
HL = 3           # heads per core (4 head groups)
D = 768
HD = 64
DL = HL * HD     # 192: my q/k/v width
DFF = 4 * D
P = 128
NDT = D // P     # 6
NTT = TLOC // P  # 16 local t-tiles
NCH = T // P     # 32 key chunks total
NCHL = TLOC // P  # 16 local key chunks
NQB = 4          # query blocks of 512
QB = TLOC // NQB  # 512
TF = 512         # FFN rows per core
NFT = DFF // P   # 24
SCALE = 1.0 / math.sqrt(D)
KVE = 2 * DL * TLOC          # 786432 elems: one core's k+v payload
A2AE = DL * TF               # 98304 elems per a2a chunk

_NC_CACHE = {}


def _build_nc():
    nc = bass.Bass(num_devices=R)
    x = nc.declare_dram_parameter("x", [TLOC, D], F32, isOutput=False)
    w_qkv = nc.declare_dram_parameter("w_qkv", [D, 3 * DL], F32, isOutput=False)
    b_qkv = nc.declare_dram_parameter("b_qkv", [3 * DL], F32, isOutput=False)
    msk = nc.declare_dram_parameter("msk", [P, 2], F32, isOutput=False)
    msk2 = nc.declare_dram_parameter("msk2", [P, 2], F32, isOutput=False)
    w_ff1 = nc.declare_dram_parameter("w_ff1", [D, DFF], F32, isOutput=False)
    b_ff1 = nc.declare_dram_parameter("b_ff1", [DFF], F32, isOutput=False)
    w_ff2 = nc.declare_dram_parameter("w_ff2", [DFF, D], F32, isOutput=False)
    b_ff2 = nc.declare_dram_parameter("b_ff2", [D], F32, isOutput=False)
    y = nc.declare_dram_parameter("y", [TF, D], F32, isOutput=True)

    from contextlib import ExitStack
    from concourse.masks import make_identity

    with tile.TileContext(nc) as tc, ExitStack() as top:
        const = top.enter_context(tc.tile_pool(name="const", bufs=1))
        dramp = top.enter_context(tc.tile_pool(name="dramp", bufs=1, space="DRAM"))

        ident = const.tile([P, P], F32, name="ident")
        make_identity(nc, ident)
        identR_dram = nc.inline_tensor(np.eye(HD + 1, dtype=np.float32),
                                       name="identR_const")
        identR = const.tile([HD + 1, HD + 1], F32R, name="identR")
        nc.sync.dma_start(identR[:], identR_dram.ap().bitcast(F32R))
        ones_dram = nc.inline_tensor(np.ones((1, P), np.float32), name="ones_const")
        ones_row = const.tile([1, P], F32R, name="ones_row")
        nc.sync.dma_start(ones_row[:], ones_dram.ap().bitcast(F32R))
        # biases: q cols 0:192, k cols 192:384, v cols 384:576 of b_qkv
        bq_q0 = const.tile([P, 1], F32, name="bq_q0")
        nc.sync.dma_start(bq_q0[:], b_qkv.ap()[0:P].rearrange("(p o) -> p o", o=1))
        bq_k0 = const.tile([P, 1], F32, name="bq_k0")
        nc.sync.dma_start(bq_k0[:], b_qkv.ap()[DL:DL + P].rearrange("(p o) -> p o", o=1))
        # head-2 biases (separate tiles so evac outputs stay base-partition 0)
        bq_tq = const.tile([HD, 1], F32, name="bq_tq")
        nc.sync.dma_start(bq_tq[:], b_qkv.ap()[P:DL].rearrange("(p o) -> p o", o=1))
        bq_tk = const.tile([HD, 1], F32, name="bq_tk")
        nc.sync.dma_start(bq_tk[:], b_qkv.ap()[DL + P:2 * DL].rearrange("(p o) -> p o", o=1))
        bv_sb = const.tile([1, DL], F32, name="bv_sb")
        nc.sync.dma_start(bv_sb[:], b_qkv.ap()[None, 2 * DL:3 * DL])
        b1_sb = const.tile([P, NFT], F32, name="b1_sb")
        nc.sync.dma_start(b1_sb[:], b_ff1.ap().rearrange("(o p) -> p o", p=P))
        b2_sb = const.tile([1, D], F32R, name="b2_sb")
        nc.sync.dma_start(b2_sb[:], b_ff2.ap()[None, :].bitcast(F32R))
        msk_sb = const.tile([P, 2], F32, name="msk_sb")
        nc.sync.dma_start(msk_sb[:], msk.ap())
        msk2_sb = const.tile([P, 2], F32, name="msk2_sb")
        nc.sync.dma_start(msk2_sb[:], msk2.ap())

        rs_in = dramp.tile([2 * KVE], BF16, name="rs_in")
        rs_out = dramp.tile([KVE], BF16, name="rs_out")
        a2a1_in = dramp.tile([R * P * TF], BF16, name="a2a1_in")
        a2a1_out = dramp.tile([R * P * TF], BF16, name="a2a1_out")
        a2a2_in = dramp.tile([R * HD * TF], BF16, name="a2a2_in")
        a2a2_out = dramp.tile([R * HD * TF], BF16, name="a2a2_out")

        # attention outputs, masked versions for the AllToAll (freed right
        # after the a2a input DMAs)
        at_scope = top.enter_context(ExitStack())
        atp = at_scope.enter_context(tc.tile_pool(name="atp", bufs=1))
        attA = [atp.tile([P, TLOC], BF16, name=f"attA{m}") for m in range(2)]
        attB = [atp.tile([HD, TLOC], BF16, name=f"attB{m}") for m in range(2)]

        kv_scope = top.enter_context(ExitStack())
        kvp = kv_scope.enter_context(tc.tile_pool(name="kvp", bufs=1))
        # q/k: heads 0,1 on 128 partitions; head 2 in separate 64-row tiles
        # (matmul lhsT/rhs must share a base partition).
        qTa = kvp.tile([P, TLOC], BF16, name="qTa")
        kTa = kvp.tile([P, TLOC], BF16, name="kTa")
        qTb = kvp.tile([HD, TLOC], BF16, name="qTb")
        kTb = kvp.tile([HD, TLOC], BF16, name="kTb")
        kTra = kvp.tile([P, TLOC], BF16, name="kTra")
        kTrb = kvp.tile([HD, TLOC], BF16, name="kTrb")
        # v chunks with a ones column per head: [128, 3, 65]. The ones
        # columns are written once up front; later v DMAs/evacs only touch
        # columns 0:64 so no mid-pipeline memsets are needed.
        vf = [kvp.tile([P, HL, HD + 1], BF16, name=f"vf{c}") for c in range(NCH)]
        for c in range(NCH):
            nc.gpsimd.memset(vf[c][:, :, HD:HD + 1], 1.0)

        # ------------------------------------------------------------------
        # Phase 1 SBUF (outlives the phase-1 PSUM scope: xT/wq feed the
        # deferred v projection, vm feeds the ReduceScatter input DMAs)
        # ------------------------------------------------------------------
        xq_scope = top.enter_context(ExitStack())
        xp = xq_scope.enter_context(tc.tile_pool(name="xp", bufs=1))
        xsp = xq_scope.enter_context(tc.tile_pool(name="xsp", bufs=4))
        wsp = xq_scope.enter_context(tc.tile_pool(name="wsp", bufs=1))
        mkp = xq_scope.enter_context(tc.tile_pool(name="mkp", bufs=1))
        vmp = xq_scope.enter_context(tc.tile_pool(name="vmp", bufs=4))

        xT = [xp.tile([P, TLOC], BF16, name=f"xT{d}") for d in range(NDT)]
        w_qkv_v = w_qkv.ap().rearrange("(o p) j -> p o j", p=P)
        wq = xp.tile([P, NDT, 3 * DL], BF16, name="wq")
        wt = xp.tile([P, NDT, P], BF16, name="wt")
        # stage the f32 -> bf16 weight cast through a half-sized buffer
        for o in range(2):
            osl = slice(3 * o, 3 * (o + 1))
            wqf = wsp.tile([P, NDT // 2, 3 * DL], F32, tag="wqf", name="wqf")
            nc.sync.dma_start(wqf[:], w_qkv_v[:, osl, :])
            nc.vector.tensor_copy(wq[:, osl, :], wqf[:])
            nc.vector.tensor_copy(wt[:, osl, 0:HD], wqf[:, :, P:DL])
            nc.vector.tensor_copy(wt[:, osl, HD:P], wqf[:, :, DL + P:2 * DL])
        ones_bf = xp.tile([1, P], BF16, name="ones_bf")
        nc.vector.tensor_copy(ones_bf[:], ones_row[:].bitcast(F32))
        bv_bf = xp.tile([1, DL], BF16, name="bv_bf")
        nc.vector.tensor_copy(bv_bf[:], bv_sb[:])

        # masked k staging (v versions stream through vmp per t-tile)
        kma = [mkp.tile([P, TLOC], BF16, name=f"kma{m}") for m in range(2)]
        kmb = [mkp.tile([HD, TLOC], BF16, name=f"kmb{m}") for m in range(2)]

        # rs_in element layout per mask half: kT [192, 2048] then v [2048, 192]
        rs_k = rs_in.rearrange("(m a b) -> m a b", m=2, b=TLOC)  # k: rows 0:192
        rs_v = rs_in.rearrange("(m a b) -> m a b", m=2, b=DL)    # v: rows 2048:4096

        # ------------------------------------------------------------------
        # Phase 1 compute: transposes interleaved with k/tail/q projections
        # (PSUM: psT banks 0-1, psQ banks 2-5)
        # ------------------------------------------------------------------
        with ExitStack() as ph1:
            psT = ph1.enter_context(tc.tile_pool(name="psT", bufs=2, space="PSUM"))
            psQ = ph1.enter_context(tc.tile_pool(name="psQ", bufs=2, space="PSUM"))

            for tq in range(NTT // 4):
                tsl = slice(QB * tq, QB * (tq + 1))
                # transposes for this 512-token quad (4-wide psum, ACT/DVE evac)
                xs = []
                for i in range(4):
                    xt_ = xsp.tile([P, D], F32, tag="xsb", name="xsb")
                    nc.sync.dma_start(
                        xt_[:], x.ap()[P * (4 * tq + i):P * (4 * tq + i + 1), :])
                    xs.append(xt_)
                for dt_ in range(NDT):
                    pst = psT.tile([P, 4 * P], F32, tag="pst", name="pst")
                    for i in range(4):
                        nc.tensor.transpose(
                            pst[:, P * i:P * (i + 1)],
                            xs[i][:, P * dt_:P * (dt_ + 1)], ident[:])
                    dst = xT[dt_][:, 4 * P * tq:4 * P * (tq + 1)]
                    if (tq * NDT + dt_) % 2 == 0:
                        nc.scalar.copy(dst, pst[:])
                    else:
                        nc.vector.tensor_copy(dst, pst[:])
                # k heads 0,1 (512-wide MMs: the ISA caps moving operands
                # at 512 elements)
                ps = psQ.tile([P, QB], F32, tag="psq", name="psq")
                for d_ in range(NDT):
                    nc.tensor.matmul(ps[:], wq[:, d_, DL:DL + P], xT[d_][:, tsl],
                                     start=(d_ == 0), stop=(d_ == NDT - 1))
                nc.scalar.activation(kTa[:, tsl], ps[:], AFT.Identity, bias=bq_k0)
                for m in range(2):
                    nc.vector.tensor_scalar_mul(kma[m][:, tsl], kTa[:, tsl],
                                                msk_sb[:, m:m + 1])
                # packed tail MM: out rows 0:64 q head2, rows 64:128 k head2
                ps = psQ.tile([P, QB], F32, tag="psq", name="psq")
                for d_ in range(NDT):
                    nc.tensor.matmul(ps[:], wt[:, d_, :], xT[d_][:, tsl],
                                     start=(d_ == 0), stop=(d_ == NDT - 1))
                nc.scalar.activation(qTb[:, tsl], ps[0:HD, :], AFT.Identity,
                                     bias=bq_tq)
                nc.scalar.activation(kTb[:, tsl], ps[HD:P, :], AFT.Identity,
                                     bias=bq_tk)
                for m in range(2):
                    nc.vector.tensor_scalar_mul(kmb[m][:, tsl], kTb[:, tsl],
                                                msk_sb[0:HD, m:m + 1])
                # q heads 0,1
                ps = psQ.tile([P, QB], F32, tag="psq", name="psq")
                for d_ in range(NDT):
                    nc.tensor.matmul(ps[:], wq[:, d_, 0:P], xT[d_][:, tsl],
                                     start=(d_ == 0), stop=(d_ == NDT - 1))
                nc.scalar.activation(qTa[:, tsl], ps[:], AFT.Identity, bias=bq_q0)
            for m in range(2):
                nc.sync.dma_start(rs_k[m, 0:P, :], kma[m][:])
                nc.sync.dma_start(rs_k[m, P:DL, :], kmb[m][:])

        # ------------------------------------------------------------------
        # Phase 2: attention. PSUM: scp banks 0-3, accp 4-5 (from the freed
        # phase-1 banks), psV 6-7 then bcp 6-7 after the v scope closes.
        # The first query block's score MMs + exps are issued ahead of the
        # deferred v projection so ACT starts as early as possible.
        # ------------------------------------------------------------------
        with ExitStack() as ph2:
            scp = ph2.enter_context(tc.tile_pool(name="scp", bufs=2, space="PSUM"))
            accp = ph2.enter_context(tc.tile_pool(name="accp", bufs=2, space="PSUM"))
            weip = ph2.enter_context(tc.tile_pool(name="weip", bufs=4))
            polp = ph2.enter_context(tc.tile_pool(name="polp", bufs=2))
            parp = ph2.enter_context(tc.tile_pool(name="parp", bufs=1))
            tailp = ph2.enter_context(tc.tile_pool(name="tailp", bufs=4))

            # cubic exp offload: wei = p(y) ~= exp(SCALE*y) for raw logits y,
            # evaluated on DVE or Pool to relieve the ACT bottleneck.
            # coeffs fit on x in [-0.75, 0.75], max rel err 0.3%.
            PC0 = 0.99915013
            PC1 = 1.0032008 * SCALE
            PC2 = 0.51870262 * SCALE * SCALE
            PC3 = 0.1600611 * SCALE * SCALE * SCALE

            def make_wei(sc, off, wpool=None):
                wei = (wpool or weip).tile([P, 2 * QB], BF16, tag="wei",
                                           name="wei")
                if off is None:
                    nc.scalar.activation(wei[:], sc[:], AFT.Exp, scale=SCALE)
                    return wei
                eng = nc.vector if off == "dve" else nc.gpsimd
                y = polp.tile([P, 2 * QB], BF16, tag="py" + off, name="py")
                nc.vector.tensor_copy(y[:], sc[:])
                t1 = polp.tile([P, 2 * QB], BF16, tag="pt" + off, name="pt")
                eng.tensor_scalar(t1[:], y[:], PC3, PC2, ALU.mult, ALU.add)
                t2 = polp.tile([P, 2 * QB], BF16, tag="pt" + off, name="pt")
                eng.tensor_tensor(t2[:], t1[:], y[:], ALU.mult)
                t3 = polp.tile([P, 2 * QB], BF16, tag="pt" + off, name="pt")
                eng.tensor_scalar(t3[:], t2[:], PC1, None, ALU.add)
                t4 = polp.tile([P, 2 * QB], BF16, tag="pt" + off, name="pt")
                eng.tensor_tensor(t4[:], t3[:], y[:], ALU.mult)
                eng.tensor_scalar(wei[:], t4[:], PC0, None, ALU.add)
                return wei

            # chunk -> engine assignment within each 16-chunk loop
            OFF = {}

            # SBUF partials: pass A 2 heads x 4 qb, pass B 1 head x 4 qb
            parA = [parp.tile([HD + 1, QB], F32R, name=f"parA{i}") for i in range(8)]
            parB = [parp.tile([HD + 1, QB], F32R, name=f"parB{i}") for i in range(4)]

            def sc_mm_a(sc, c, qsl):
                """pass A scores for chunk c: heads 0,1 side by side."""
                ka = kTa if c < NCHL else kTra
                cc = c % NCHL
                ksl = slice(P * cc, P * (cc + 1))
                nc.tensor.matmul(sc[:, 0:QB], ka[0:HD, ksl], qTa[0:HD, qsl],
                                 start=True, stop=True)
                nc.tensor.matmul(sc[:, QB:2 * QB], ka[HD:P, ksl],
                                 qTa[HD:P, qsl], start=True, stop=True)

            def sc_mm_b(sc, c, qsl2):
                """pass B scores for chunk c: head 2, a 1024-query pair
                (two 512-col MMs: the ISA caps moving operands at 512)."""
                cc = c % NCHL
                ksl = slice(P * cc, P * (cc + 1))
                kb = kTb if c < NCHL else kTrb
                q0, q1 = qsl2.start, qsl2.start + QB
                nc.tensor.matmul(sc[:, 0:QB], kb[:, ksl], qTb[:, q0:q0 + QB],
                                 start=True, stop=True)
                nc.tensor.matmul(sc[:, QB:2 * QB], kb[:, ksl], qTb[:, q1:q1 + QB],
                                 start=True, stop=True)

            # ---- deferred v projection scope (psV banks 6-7). The first
            # 12 of qb0's score/exp pairs are hoisted ahead of the v
            # projection so ACT starts as early as possible. ----
            NH0 = 12
            with ExitStack() as phV:
                weip0 = phV.enter_context(tc.tile_pool(name="weip0", bufs=NH0))
                psV = phV.enter_context(
                    tc.tile_pool(name="psV", bufs=2, space="PSUM"))

                qsl0 = slice(0, QB)
                wei0 = []
                for c in range(NH0):
                    sc = scp.tile([P, 2 * QB], F32, tag="sc", name="sc")
                    sc_mm_a(sc, c, qsl0)
                    w_ = weip0.tile([P, 2 * QB], BF16, tag="wei0", name="wei0")
                    nc.scalar.activation(w_[:], sc[:], AFT.Exp, scale=SCALE)
                    wei0.append(w_)
                for tt in range(NTT):
                    ps = psV.tile([P, DL], F32, tag="psv", name="psv")
                    for d_ in range(NDT):
                        nc.tensor.matmul(ps[:], xT[d_][:, P * tt:P * (tt + 1)],
                                         wq[:, d_, 2 * DL:3 * DL],
                                         start=(d_ == 0), stop=False)
                    nc.tensor.matmul(ps[:], ones_bf[:], bv_bf[:],
                                     start=False, stop=True)
                    dst = vf[tt]
                    nc.vector.tensor_copy(
                        dst[:, :, 0:HD],
                        ps[:].rearrange("p (h e) -> p h e", e=HD))
                    for m in range(2):
                        vmt = vmp.tile([P, DL], BF16, tag="vmt", name="vmt")
                        nc.vector.tensor_scalar_mul(vmt[:], ps[:],
                                                    msk_sb[:, m:m + 1])
                        nc.sync.dma_start(
                            rs_v[m, TLOC + P * tt:TLOC + P * (tt + 1), :], vmt[:])

                nc.gpsimd.collective_compute(
                    "ReduceScatter", ALU.add,
                    replica_groups=[[0, 1], [2, 3], [4, 5], [6, 7]],
                    ins=[rs_in[:]], outs=[rs_out[:]],
                )

                # unpack partner kv from rs_out (ones-column memsets for the
                # remote vf tiles are deferred to just before the remote
                # passes: on the in-order Pool queue they would otherwise
                # block the local poly-exp chains behind the collective)
                ro_k = rs_out.rearrange("(a b) -> a b", b=TLOC)
                ro_v = rs_out.rearrange("(a b) -> a b", b=DL)
                nc.sync.dma_start(kTra[:], ro_k[0:P, :])
                nc.sync.dma_start(kTrb[:], ro_k[P:DL, :])
                for c2 in range(NCHL):
                    dst = vf[NCHL + c2]
                    nc.sync.dma_start(
                        dst[:, :, 0:HD],
                        ro_v[TLOC + P * c2:TLOC + P * (c2 + 1), :].rearrange(
                            "a (h e) -> a h e", e=HD))

                # chunk scan with deferred offloaded-chunk accumulation:
                # the Pool/DVE poly chains get ~10 chunks of slack before
                # the in-order PE queue consumes their wei, avoiding stalls
                def chunk_scan(crange, get_sc, vh0, vh1, acc0, acc1,
                               opening, hook=None):
                    deferred = []
                    issued = False
                    for idx, c in enumerate(crange):
                        sc = scp.tile([P, 2 * QB], F32, tag="sc", name="sc")
                        get_sc(sc, c)
                        off = OFF.get(c % NCHL)
                        wei = make_wei(sc, off)
                        if off is not None:
                            deferred.append((c, wei))
                        else:
                            st = opening and not issued
                            issued = True
                            nc.tensor.matmul(acc0[:], vf[c][:, vh0, :],
                                             wei[:, 0:QB], start=st, stop=False)
                            nc.tensor.matmul(acc1[:], vf[c][:, vh1, :],
                                             wei[:, QB:2 * QB],
                                             start=st, stop=False)
                        if hook is not None and idx == 3:
                            hook()
                    for j, (c, wei) in enumerate(deferred):
                        last = (j == len(deferred) - 1)
                        nc.tensor.matmul(acc0[:], vf[c][:, vh0, :],
                                         wei[:, 0:QB], start=False, stop=last)
                        nc.tensor.matmul(acc1[:], vf[c][:, vh1, :],
                                         wei[:, QB:2 * QB],
                                         start=False, stop=last)

                # ---- pass A local qb0: consume hoisted weis, finish rest --
                acc0 = accp.tile([HD + 1, QB], F32, tag="acc", name="acc0")
                acc1 = accp.tile([HD + 1, QB], F32, tag="acc", name="acc1")
                for c in range(NCHL):
                    if c < NH0:
                        w_ = wei0[c]
                    else:
                        sc = scp.tile([P, 2 * QB], F32, tag="sc", name="sc")
                        sc_mm_a(sc, c, qsl0)
                        w_ = weip.tile([P, 2 * QB], BF16, tag="wei", name="wei")
                        nc.scalar.activation(w_[:], sc[:], AFT.Exp, scale=SCALE)
                    nc.tensor.matmul(acc0[:], vf[c][:, 0, :], w_[:, 0:QB],
                                     start=(c == 0), stop=(c == NCHL - 1))
                    nc.tensor.matmul(acc1[:], vf[c][:, 1, :], w_[:, QB:2 * QB],
                                     start=(c == 0), stop=(c == NCHL - 1))
                nc.vector.tensor_copy(parA[0][:], acc0[:])
                nc.vector.tensor_copy(parA[1][:], acc1[:])

                # ---- pass A local qb1..3 ----
                for qb in range(1, NQB):
                    qsl = slice(QB * qb, QB * (qb + 1))
                    acc0 = accp.tile([HD + 1, QB], F32, tag="acc", name="acc0")
                    acc1 = accp.tile([HD + 1, QB], F32, tag="acc", name="acc1")
                    chunk_scan(range(NCHL), lambda sc, c: sc_mm_a(sc, c, qsl),
                               0, 1, acc0, acc1, opening=True)
                    nc.vector.tensor_copy(parA[2 * qb][:], acc0[:])
                    nc.vector.tensor_copy(parA[2 * qb + 1][:], acc1[:])

                # ---- pass B local: head 2, 1024-query pairs ----
                for qp in range(2):
                    qsl2 = slice(2 * QB * qp, 2 * QB * (qp + 1))
                    acc0 = accp.tile([HD + 1, QB], F32, tag="acc", name="acc0")
                    acc1 = accp.tile([HD + 1, QB], F32, tag="acc", name="acc1")
                    chunk_scan(range(NCHL), lambda sc, c: sc_mm_b(sc, c, qsl2),
                               2, 2, acc0, acc1, opening=True)
                    nc.vector.tensor_copy(parB[2 * qp][:], acc0[:])
                    nc.vector.tensor_copy(parB[2 * qp + 1][:], acc1[:])

            # v-projection PSUM freed; bcp takes banks 6-7
            bc_scope = ExitStack()
            bcp = bc_scope.enter_context(
                tc.tile_pool(name="bcp", bufs=2, space="PSUM"))

            pending = []

            def finish_evac(acc, hh, qsl, width):
                """evacuate acc to SBUF (frees the PSUM bank + starts the
                reciprocal); the PE-side normalize is deferred via pending."""
                fin = tailp.tile([HD + 1, width], F32, tag="fin", name="fin")
                nc.vector.tensor_copy(fin[:], acc[:])
                recip = tailp.tile([1, width], F32R, tag="recip", name="recip")
                with nc.allow_low_precision(reason="f32r is bit-identical to f32"):
                    nc.vector.reciprocal(recip[:], fin[HD:HD + 1, :].bitcast(F32R))
                pending.append((fin, recip, hh, qsl, width))

            def flush_finish():
                for fin, recip, hh, qsl, width in pending:
                    bc = bcp.tile([HD, width], F32, tag="bc", name="bc")
                    nc.tensor.matmul(bc[:], ones_row[:, 0:HD], recip[:],
                                     start=True, stop=True)
                    rec0 = tailp.tile([HD, width], F32, tag="recb", name="rec0")
                    rec1 = tailp.tile([HD, width], F32, tag="recb", name="rec1")
                    nc.vector.tensor_scalar_mul(rec0[:], bc[:], msk2_sb[0:HD, 0:1])
                    nc.vector.tensor_scalar_mul(rec1[:], bc[:], msk2_sb[0:HD, 1:2])
                    for m, rec in ((0, rec0), (1, rec1)):
                        if hh < 2:
                            dst = attA[m][HD * hh:HD * (hh + 1), qsl]
                        else:
                            dst = attB[m][:, qsl]
                        nc.vector.tensor_tensor(dst, fin[0:HD, :].bitcast(F32R),
                                                rec[:].bitcast(F32R), ALU.mult)
                pending.clear()

            # ---- pass A remote + finish ----
            id65 = identR[:]
            for qb in range(NQB):
                qsl = slice(QB * qb, QB * (qb + 1))
                acc0 = accp.tile([HD + 1, QB], F32, tag="acc", name="acc0")
                acc1 = accp.tile([HD + 1, QB], F32, tag="acc", name="acc1")
                nc.tensor.matmul(acc0[:], id65, parA[2 * qb][:],
                                 start=True, stop=False)
                nc.tensor.matmul(acc1[:], id65, parA[2 * qb + 1][:],
                                 start=True, stop=False)
                chunk_scan(range(NCHL, NCH), lambda sc, c: sc_mm_a(sc, c, qsl),
                           0, 1, acc0, acc1, opening=False, hook=flush_finish)
                finish_evac(acc0, 0, qsl, QB)
                finish_evac(acc1, 1, qsl, QB)

            # ---- pass B remote + finish ----
            for qp in range(2):
                qsl2 = slice(2 * QB * qp, 2 * QB * (qp + 1))
                acc0 = accp.tile([HD + 1, QB], F32, tag="acc", name="acc0")
                acc1 = accp.tile([HD + 1, QB], F32, tag="acc", name="acc1")
                nc.tensor.matmul(acc0[:], id65, parB[2 * qp][:],
                                 start=True, stop=False)
                nc.tensor.matmul(acc1[:], id65, parB[2 * qp + 1][:],
                                 start=True, stop=False)
                chunk_scan(range(NCHL, NCH), lambda sc, c: sc_mm_b(sc, c, qsl2),
                           2, 2, acc0, acc1, opening=False, hook=flush_finish)
                finish_evac(acc0, 2, slice(2 * QB * qp, 2 * QB * qp + QB), QB)
                finish_evac(acc1, 2, slice(2 * QB * qp + QB, 2 * QB * (qp + 1)), QB)
            flush_finish()
            bc_scope.close()

        xq_scope.close()
        kv_scope.close()

        # ------------------------------------------------------------------
        # Phase 2b: AllToAll redistribution of attention rows, split by
        # head-rows: the heads-0,1 collective (2/3 of the payload) only
        # needs attA, which completes a pass earlier than attB, so it hides
        # under the pass-B attention tail. Only the small head-2 collective
        # is exposed. The w_ff1 preload is issued right after the (tiny)
        # a2a input DMAs so its transfer runs under the collectives.
        # ------------------------------------------------------------------
        a2a1_v = a2a1_in.rearrange("(d a b) -> d a b", d=R, b=TF)
        a2a2_v = a2a2_in.rearrange("(d a b) -> d a b", d=R, b=TF)
        for d in range(R):
            qsl = slice(TF * (d // 2), TF * (d // 2 + 1))
            m = d % 2
            nc.sync.dma_start(a2a1_v[d, :, :], attA[m][:, qsl])
        nc.gpsimd.collective_compute(
            "AllToAll", ALU.bypass,
            replica_groups=[list(range(R))],
            ins=[a2a1_in[:]], outs=[a2a1_out[:]],
        )
        for d in range(R):
            qsl = slice(TF * (d // 2), TF * (d // 2 + 1))
            m = d % 2
            nc.sync.dma_start(a2a2_v[d, :, :], attB[m][:, qsl])
        nc.gpsimd.collective_compute(
            "AllToAll", ALU.bypass,
            replica_groups=[list(range(R))],
            ins=[a2a2_in[:]], outs=[a2a2_out[:]],
        )
        at_scope.close()

        # assemble attnT_ffn [768, 512] f32r from chunk pairs
        ffp = top.enter_context(tc.tile_pool(name="ffp", bufs=1))
        attnT = [ffp.tile([P, TF], F32R, name=f"attnT{i}") for i in range(NDT)]
        hTp = top.enter_context(tc.tile_pool(name="hTp", bufs=1))
        hT = [hTp.tile([P, TF], F32R, name=f"hT{f}") for f in range(NFT)]

        w1p = top.enter_context(tc.tile_pool(name="w1p", bufs=1))
        w1 = w1p.tile([P, NDT, DFF], F32R, name="w1")
        w_ff1_v = w_ff1.ap().rearrange("(o p) f -> p o f", p=P)
        # fine-grained split so the small a2a input DMAs can interleave
        # with this 9.4MB preload on the DMA engines
        for d_ in range(NDT):
            for hf in range(4):
                fsl = slice(DFF // 4 * hf, DFF // 4 * (hf + 1))
                nc.sync.dma_start(w1[:, d_, fsl],
                                  w_ff1_v[:, d_, fsl].bitcast(F32R))

        ao1_v = a2a1_out.rearrange("(j a b) -> j a b", j=R, b=TF)
        ao2_v = a2a2_out.rearrange("(j a b) -> j a b", j=R, b=TF)
        with ExitStack() as ph2b:
            rxp = ph2b.enter_context(tc.tile_pool(name="rxp", bufs=1))
            chA = [rxp.tile([P, TF], BF16, name=f"chA{g}") for g in range(4)]
            chB = [rxp.tile([HD, TF], BF16, name=f"chB{g}") for g in range(4)]
            chA2 = [rxp.tile([P, TF], BF16, name=f"chA2{g}") for g in range(4)]
            chB2 = [rxp.tile([HD, TF], BF16, name=f"chB2{g}") for g in range(4)]
            for g in range(4):
                nc.sync.dma_start(chA[g][:], ao1_v[2 * g, :, :])
                nc.sync.dma_start(chA2[g][:], ao1_v[2 * g + 1, :, :])
            for g in range(4):
                nc.sync.dma_start(chB[g][:], ao2_v[2 * g, :, :])
                nc.sync.dma_start(chB2[g][:], ao2_v[2 * g + 1, :, :])
            # 12 blocks of 64 rows: block b = global dims 64b..64b+64; the
            # s-parity masking guarantees exactly one addend is nonzero.
            # chA-sourced blocks first: their data lands a collective earlier
            for b in [b for b in range(12) if b % 3 < 2] + \
                     [b for b in range(12) if b % 3 == 2]:
                g, rsub = b // 3, b % 3
                if rsub < 2:
                    s0 = chA[g][HD * rsub:HD * (rsub + 1), :]
                    s1 = chA2[g][HD * rsub:HD * (rsub + 1), :]
                else:
                    s0, s1 = chB[g][:], chB2[g][:]
                dst = attnT[b // 2][HD * (b % 2):HD * (b % 2 + 1), :]
                with nc.allow_low_precision(reason="one addend is zero"):
                    nc.vector.tensor_tensor(dst, s0, s1, ALU.add)

        # ------------------------------------------------------------------
        # Phase 3a: FFN1  h^T[f, t] = gelu(W1^T attn^T + b1)
        # ------------------------------------------------------------------
        with ExitStack() as ph3:
            ps1 = ph3.enter_context(tc.tile_pool(name="ps1", bufs=2, space="PSUM"))
            for ft in range(NFT):
                ps = ps1.tile([P, TF], F32, tag="ps1t", name="ps1t")
                for d_ in range(NDT):
                    nc.tensor.matmul(ps[:], w1[:, d_, P * ft:P * (ft + 1)],
                                     attnT[d_][:],
                                     start=(d_ == 0), stop=(d_ == NDT - 1))
                nc.scalar.activation(hT[ft][:], ps[:], AFT.Gelu,
                                     bias=b1_sb[:, ft:ft + 1])

        # ------------------------------------------------------------------
        # Phase 3b: FFN2  out[t, o] = h^T^T W2 + b2 (deep w2 prefetch so
        # half the weights stream in during FFN1)
        # ------------------------------------------------------------------
        w_ff2_v = w_ff2.ap().rearrange("(o p) d -> p o d", p=P)
        with ExitStack() as ph4:
            w2p = ph4.enter_context(tc.tile_pool(name="w2p", bufs=12))
            ps2 = ph4.enter_context(tc.tile_pool(name="ps2", bufs=1, space="PSUM"))
            outp = ph4.enter_context(tc.tile_pool(name="outp", bufs=1))
            acc2 = [ps2.tile([P, 384], F32, name=f"acc2_{g}") for g in range(8)]
            for ft in range(NFT):
                w2 = w2p.tile([P, D], F32R, tag="w2", name="w2")
                nc.sync.dma_start(w2[:], w_ff2_v[:, ft, :].bitcast(F32R))
                for tt in range(TF // P):
                    for o2 in range(2):
                        g = tt * 2 + o2
                        nc.tensor.matmul(acc2[g][:],
                                         hT[ft][:, P * tt:P * (tt + 1)],
                                         w2[:, 384 * o2:384 * (o2 + 1)],
                                         start=(ft == 0), stop=False)
            out_sb = [outp.tile([P, D], F32, name=f"out{tt}") for tt in range(TF // P)]
            for tt in range(TF // P):
                for o2 in range(2):
                    g = tt * 2 + o2
                    sl = slice(384 * o2, 384 * (o2 + 1))
                    nc.tensor.matmul(acc2[g][:], ones_row[:], b2_sb[:, sl],
                                     start=False, stop=True)
                    nc.vector.tensor_copy(out_sb[tt][:, sl], acc2[g][:])
                nc.sync.dma_start(y.ap()[P * tt:P * (tt + 1), :], out_sb[tt][:])

    return nc


def _get_nc():
    if "nc" not in _NC_CACHE:
        _NC_CACHE["nc"] = _build_nc()
    return _NC_CACHE["nc"]


def run_sharded(inputs, **run_kwargs):
    """Run the SPMD kernel; returns (full_output [1,4096,768], BassKernelResults)."""
    x = np.ascontiguousarray(np.asarray(inputs["x"], dtype=np.float32))
    assert x.shape == (1, T, D), x.shape
    w_qkv = np.asarray(inputs["w_qkv"], dtype=np.float32)
    b_qkv = np.asarray(inputs["b_qkv"], dtype=np.float32)
    common = {}
    for nm in ("w_ff1", "b_ff1", "w_ff2", "b_ff2"):
        common[nm] = np.ascontiguousarray(np.asarray(inputs[nm], dtype=np.float32))
    in_maps = []
    for c in range(R):
        h, s = c // 2, c % 2
        m = dict(common)
        m["x"] = np.ascontiguousarray(x[0, TLOC * s:TLOC * (s + 1), :])
        cols = np.concatenate([
            np.arange(DL * h, DL * (h + 1)),
            D + np.arange(DL * h, DL * (h + 1)),
            2 * D + np.arange(DL * h, DL * (h + 1)),
        ])
        m["w_qkv"] = np.ascontiguousarray(w_qkv[:, cols])
        m["b_qkv"] = np.ascontiguousarray(b_qkv[cols])
        m["msk"] = np.ascontiguousarray(
            np.tile(np.array([[s, 1 - s]], np.float32), (P, 1)))
        m["msk2"] = np.ascontiguousarray(
            np.tile(np.array([[1 - s, s]], np.float32), (P, 1)))
        in_maps.append(m)
    nc = _get_nc()
    res = run_bass_kernel_spmd(nc, in_maps, core_ids=list(range(R)), **run_kwargs)
    out = np.zeros((T, D), np.float32)
    for c in range(R):
        h, s = c // 2, c % 2
        r0 = TLOC * s + TF * h
        out[r0:r0 + TF, :] = res.results[c]["y"]
    return out.reshape(1, T, D), res


def kernel(**inputs):
    out, _ = run_sharded(inputs)
    return out
